# revision 4
# baseline (speedup 1.0000x reference)
"""Trainium2 Bass kernel for nn_EuclideanEmbedding (fused cutoff-multiply +
segment_sum over 3.2M edges into 100k nodes, 16 features).

Strategy
--------
Host: counting-sort edges by receiver, shard nodes across 8 cores
(12512 nodes/core), pad edges into fixed-capacity 32-node buckets
(CB chunks of 128 edge-slots each), lay out tiles chunk-major.

Device (per core): for each 128-edge chunk, build a one-hot selection
matrix sel[e, j] = (receiver_local[e] == j) on the vector engine and use
the tensor engine (PE) to compute selT @ (senders * cutoff(lengths)),
accumulating each bucket's chunks into a PSUM partition window
(offsets 0/32/64 -> 96-node groups). Drain groups to SBUF, one DMA out.

Output rows >= 100000 of the full [3.2M, 16] result are identically zero
(receivers < 100000), assembled host-side.
"""
import math

import numpy as np

E = 3_200_000
F = 16
N_NODES = 100_000
R_CUT = 5.0
INV_AVG = 1.0 / 32.0

N_CORES = 8
W = 32                      # nodes per bucket
BPG = 3                     # buckets per PSUM group (96 nodes, offsets 0/32/64)
NODES_PER_CORE = 12_512     # 391 real buckets per core
RBUCKETS = NODES_PER_CORE // W          # 391
BUCKETS_CORE = 393                      # padded bucket slots (131 groups * 3)
GROUPS = BUCKETS_CORE // BPG            # 131
NODES_PAD = BUCKETS_CORE * W            # 12576
N_RBUCKETS_TOT = (N_NODES + W - 1) // W  # 3125 real buckets globally

_CACHE = {}


def _build_program(cb: int, sel_bf16: bool):
    """Build the Bass/Tile program for chunks-per-bucket `cb`."""
    from contextlib import ExitStack

    import concourse.bacc as bacc
    import concourse.tile as tile
    from concourse import mybir

    T = BPG * cb                 # chunks per group
    nchunks = GROUPS * T
    sel_dt = mybir.dt.bfloat16 if sel_bf16 else mybir.dt.float32

    nc = bacc.Bacc("TRN2", target_bir_lowering=False, debug=False,
                   enable_asserts=False, num_devices=N_CORES)
    x_dram = nc.dram_tensor("x_t", [GROUPS, 128, T * 16], mybir.dt.float32,
                            kind="ExternalInput").ap()
    len_dram = nc.dram_tensor("len_t", [128, nchunks], mybir.dt.float32,
                              kind="ExternalInput").ap()
    rkey_dram = nc.dram_tensor("rkey_t", [128, nchunks], mybir.dt.float32,
                               kind="ExternalInput").ap()
    iota_dram = nc.dram_tensor("iota32", [128, W], mybir.dt.float32,
                               kind="ExternalInput").ap()
    out_dram = nc.dram_tensor("out", [NODES_PAD, 16], mybir.dt.float32,
                              kind="ExternalOutput").ap()

    with tile.TileContext(nc) as tc, ExitStack() as ctx:
        small = ctx.enter_context(tc.tile_pool(name="small", bufs=1))
        xin = ctx.enter_context(tc.tile_pool(name="xin", bufs=3))
        work = ctx.enter_context(tc.tile_pool(name="work", bufs=2))
        psum = ctx.enter_context(tc.tile_pool(name="psum", bufs=4, space="PSUM"))

        leng = small.tile([128, nchunks], mybir.dt.float32)
        rkey = small.tile([128, nchunks], mybir.dt.float32)
        iota = small.tile([128, W], mybir.dt.float32)
        nc.sync.dma_start(leng[:], len_dram[:])
        nc.sync.dma_start(rkey[:], rkey_dram[:])
        nc.sync.dma_start(iota[:], iota_dram[:])

        u = small.tile([128, nchunks], mybir.dt.float32)
        msk = small.tile([128, nchunks], mybir.dt.float32)
        w = small.tile([128, nchunks], mybir.dt.float32)
        halfpi = small.tile([128, 1], mybir.dt.float32)
        nc.gpsimd.memset(halfpi[:], math.pi / 2)
        # u = sin(pi/2 - (pi/R_CUT) r) = cos(pi r / R_CUT); arg in [-pi, pi]
        nc.scalar.activation(u[:], leng[:], mybir.ActivationFunctionType.Sin,
                             bias=halfpi[:, 0:1], scale=-math.pi / R_CUT)
        nc.vector.tensor_scalar(msk[:], leng[:], R_CUT, None,
                                mybir.AluOpType.is_lt)
        nc.vector.tensor_scalar(w[:], u[:], 1.0, 0.5 * INV_AVG,
                                mybir.AluOpType.add, mybir.AluOpType.mult)
        nc.vector.tensor_tensor(w[:], w[:], msk[:], mybir.AluOpType.mult)

        out_sbuf = small.tile([128, GROUPS * 16], mybir.dt.float32)

        for t in range(GROUPS):
            xt = xin.tile([128, T * 16], mybir.dt.float32)
            nc.sync.dma_start(xt[:], x_dram[t])

            scaled = work.tile([128, T * 16], sel_dt, tag="scaled")
            nc.vector.tensor_tensor(
                scaled[:].rearrange("p (c f) -> p c f", f=16),
                xt[:].rearrange("p (c f) -> p c f", f=16),
                w[:, t * T:(t + 1) * T].unsqueeze(2).broadcast_to([128, T, 16]),
                mybir.AluOpType.mult)

            sel = work.tile([128, T * W], sel_dt, tag="sel")
            nc.vector.tensor_tensor(
                sel[:].rearrange("p (c j) -> p c j", j=W),
                rkey[:, t * T:(t + 1) * T].unsqueeze(2).broadcast_to([128, T, W]),
                iota[:].unsqueeze(1).broadcast_to([128, T, W]),
                mybir.AluOpType.is_equal)

            pt = psum.tile([128, 16], mybir.dt.float32)
            for c in range(T):
                b = c // cb
                ph = c % cb
                nc.tensor.matmul(
                    out=pt[32 * b:32 * b + W, :],
                    lhsT=sel[:, W * c:W * (c + 1)],
                    rhs=scaled[:, 16 * c:16 * (c + 1)],
                    start=(ph == 0), stop=(ph == cb - 1))
            nc.scalar.copy(out_sbuf[:96, 16 * t:16 * (t + 1)], pt[:96, :])

        nc.sync.dma_start(
            out_dram.rearrange("(g p) f -> p g f", p=96),
            out_sbuf[:96].rearrange("p (g f) -> p g f", f=16))

    nc.compile()
    return nc


def _prepare_inputs(senders, lengths, receivers, cb: int):
    """Counting-sort + bucket-pad + tile-transpose. Returns in_maps (8 dicts)."""
    cap = cb * 128
    T = BPG * cb
    nchunks = GROUPS * T

    recv = np.ascontiguousarray(np.asarray(receivers).astype(np.int64))
    order = np.argsort(recv, kind="stable").astype(np.int64)
    rs = recv[order]                         # sorted receivers
    rk_sorted = (rs % W).astype(np.float32)  # node-in-bucket key
    gbucket_counts = np.bincount((rs // W).astype(np.int64),
                                 minlength=N_RBUCKETS_TOT)
    starts = np.concatenate([[0], np.cumsum(gbucket_counts)[:-1]])

    senders_ext = np.concatenate(
        [np.asarray(senders, dtype=np.float32), np.zeros((1, F), np.float32)])
    len_ext = np.concatenate(
        [np.asarray(lengths, dtype=np.float32).reshape(-1),
         np.full(1, 6.0, np.float32)])
    rk_ext = np.concatenate([rk_sorted, np.zeros(1, np.float32)])

    iota32 = np.tile(np.arange(W, dtype=np.float32), (128, 1))
    arange_cap = np.arange(cap, dtype=np.int64)

    in_maps = []
    for k in range(N_CORES):
        bidx = RBUCKETS * k + np.arange(BUCKETS_CORE)
        slot_real = (np.arange(BUCKETS_CORE) < RBUCKETS) & (bidx < N_RBUCKETS_TOT)
        cnt = np.where(slot_real, gbucket_counts[np.minimum(bidx, N_RBUCKETS_TOT - 1)], 0)
        st = np.where(slot_real, starts[np.minimum(bidx, N_RBUCKETS_TOT - 1)], 0)
        if cnt.max() > cap:
            raise ValueError(f"bucket overflow: {cnt.max()} > {cap}")
        src = st[:, None] + arange_cap[None, :]          # [393, cap] sorted pos
        valid = arange_cap[None, :] < cnt[:, None]
        srcc = np.minimum(src, E - 1)
        edge_ids = np.where(valid, order[srcc], E)       # E -> pad row
        sort_ids = np.where(valid, srcc, E)              # for sorted-keyed arrays

        x_pad = senders_ext[edge_ids.reshape(-1)]        # [393*cap, 16]
        l_pad = len_ext[edge_ids.reshape(-1)]            # pad row E -> 6.0
        r_pad = rk_ext[sort_ids.reshape(-1)]             # pad row E -> 0.0

        x_t = x_pad.reshape(GROUPS, T, 128, 16).transpose(0, 2, 1, 3).reshape(
            GROUPS, 128, T * 16)
        len_t = np.ascontiguousarray(l_pad.reshape(nchunks, 128).T)
        rkey_t = np.ascontiguousarray(r_pad.reshape(nchunks, 128).T)
        in_maps.append({
            "x_t": np.ascontiguousarray(x_t, dtype=np.float32),
            "len_t": len_t.astype(np.float32),
            "rkey_t": rkey_t.astype(np.float32),
            "iota32": iota32,
        })
    return in_maps


def _get_program(cb: int, sel_bf16: bool):
    key = (cb, sel_bf16)
    if key not in _CACHE:
        _CACHE[key] = _build_program(cb, sel_bf16)
    return _CACHE[key]


def _run(inputs, cb=10, sel_bf16=False, trace=False, **run_kwargs):
    from concourse.bass_utils import run_bass_kernel_spmd

    in_maps = _prepare_inputs(inputs["senders"], inputs["lengths"],
                              inputs["receivers"], cb)
    nc = _get_program(cb, sel_bf16)
    res = run_bass_kernel_spmd(nc, in_maps, core_ids=list(range(N_CORES)),
                               trace=trace, **run_kwargs)
    out_full = np.zeros((E, F), np.float32)
    for k in range(N_CORES):
        nk = min(NODES_PER_CORE, N_NODES - NODES_PER_CORE * k)
        if nk <= 0:
            continue
        out_full[NODES_PER_CORE * k:NODES_PER_CORE * k + nk] = \
            res.results[k]["out"][:nk]
    return out_full, res


def kernel(senders, lengths, vectors, receivers):
    out, _ = _run({"senders": senders, "lengths": lengths,
                   "receivers": receivers})
    return out


# revision 12
# speedup vs baseline: 29.8126x; 29.8126x over previous
"""Trainium2 Bass kernel for nn_EuclideanEmbedding (fused cutoff-multiply +
segment_sum over 3.2M edges into 100k nodes, 16 features).

Strategy
--------
Host: counting-sort edges by receiver, shard nodes across 8 cores
(12512 nodes/core), pad edges into fixed-capacity 32-node buckets
(CB chunks of 128 edge-slots each), lay out tiles chunk-major.

Device (per core): for each 128-edge chunk, build a one-hot selection
matrix sel[e, j] = (receiver_local[e] == j) on the vector engine and use
the tensor engine (PE) to compute selT @ (senders * cutoff(lengths)),
accumulating each bucket's chunks into a PSUM partition window
(offsets 0/32/64 -> 96-node groups). Drain groups to SBUF, one DMA out.

Output rows >= 100000 of the full [3.2M, 16] result are identically zero
(receivers < 100000), assembled host-side.
"""
import math

import numpy as np

E = 3_200_000
F = 16
N_NODES = 100_000
R_CUT = 5.0
INV_AVG = 1.0 / 32.0

N_CORES = 8
W = 32                      # nodes per bucket
BPG = 3                     # buckets per PSUM group (96 nodes, offsets 0/32/64)
NODES_PER_CORE = 12_512     # 391 real buckets per core
RBUCKETS = NODES_PER_CORE // W          # 391
BUCKETS_CORE = 393                      # padded bucket slots (131 groups * 3)
GROUPS = BUCKETS_CORE // BPG            # 131
NODES_PAD = BUCKETS_CORE * W            # 12576
N_RBUCKETS_TOT = (N_NODES + W - 1) // W  # 3125 real buckets globally

_CACHE = {}


def _build_program(cb: int, mode: str, reps: int = 1):
    """Build the Bass/Tile program for chunks-per-bucket `cb`.

    mode: "f32" (exact, slow), "bf16" (fast, ~2e-3), "comp" (bf16 hi+lo
    error-compensated split, ~1e-5, ~1.3x bf16 cost).
    reps > 1 repeats the whole computation (same inputs/outputs) inside one
    NEFF — used only for timing (amortizes the axon dispatch floor)."""
    from contextlib import ExitStack

    import concourse.bacc as bacc
    import concourse.tile as tile
    from concourse import mybir

    T = BPG * cb                 # chunks per group
    nchunks = GROUPS * T
    sel_dt = mybir.dt.float32 if mode == "f32" else mybir.dt.bfloat16

    nc = bacc.Bacc("TRN2", target_bir_lowering=False, debug=False,
                   enable_asserts=False, num_devices=N_CORES)
    x_dram = nc.dram_tensor("x_t", [GROUPS, 128, T * 16], mybir.dt.float32,
                            kind="ExternalInput").ap()
    len_dram = nc.dram_tensor("len_t", [128, nchunks], mybir.dt.float32,
                              kind="ExternalInput").ap()
    rkey_dram = nc.dram_tensor("rkey_t", [128, nchunks], mybir.dt.float32,
                               kind="ExternalInput").ap()
    iota_dram = nc.dram_tensor("iota32", [128, W], mybir.dt.float32,
                               kind="ExternalInput").ap()
    out_dram = nc.dram_tensor("out", [NODES_PAD, 16], mybir.dt.float32,
                              kind="ExternalOutput").ap()

    with tile.TileContext(nc) as tc, ExitStack() as ctx:
        small = ctx.enter_context(tc.tile_pool(name="small", bufs=1))
        xin = ctx.enter_context(tc.tile_pool(name="xin", bufs=3))
        work = ctx.enter_context(tc.tile_pool(name="work", bufs=2))
        psum = ctx.enter_context(tc.tile_pool(name="psum", bufs=4, space="PSUM"))

        leng = small.tile([128, nchunks], mybir.dt.float32)
        rkey = small.tile([128, nchunks], mybir.dt.float32)
        iota = small.tile([128, W], mybir.dt.float32)
        nc.sync.dma_start(leng[:], len_dram[:])
        nc.sync.dma_start(rkey[:], rkey_dram[:])
        nc.sync.dma_start(iota[:], iota_dram[:])

        u = small.tile([128, nchunks], mybir.dt.float32)
        msk = small.tile([128, nchunks], mybir.dt.float32)
        w = small.tile([128, nchunks], mybir.dt.float32)
        halfpi = small.tile([128, 1], mybir.dt.float32)
        nc.gpsimd.memset(halfpi[:], math.pi / 2)
        # u = sin(pi/2 - (pi/R_CUT) r) = cos(pi r / R_CUT); arg in [-pi, pi]
        nc.scalar.activation(u[:], leng[:], mybir.ActivationFunctionType.Sin,
                             bias=halfpi[:, 0:1], scale=-math.pi / R_CUT)
        nc.vector.tensor_scalar(msk[:], leng[:], R_CUT, None,
                                mybir.AluOpType.is_lt)
        nc.vector.tensor_scalar(w[:], u[:], 1.0, 0.5 * INV_AVG,
                                mybir.AluOpType.add, mybir.AluOpType.mult)
        nc.vector.tensor_tensor(w[:], w[:], msk[:], mybir.AluOpType.mult)

        out_sbuf = small.tile([128, GROUPS * 16], mybir.dt.float32)

        for _rep in range(reps):
         for t in range(GROUPS):
            xt = xin.tile([128, T * 16], mybir.dt.float32)
            nc.sync.dma_start(xt[:], x_dram[t])

            w_bc = w[:, t * T:(t + 1) * T].unsqueeze(2).broadcast_to([128, T, 16])
            if mode == "comp":
                sc32 = work.tile([128, T * 16], mybir.dt.float32, tag="sc32")
                nc.vector.tensor_tensor(
                    sc32[:].rearrange("p (c f) -> p c f", f=16),
                    xt[:].rearrange("p (c f) -> p c f", f=16),
                    w_bc, mybir.AluOpType.mult)
                hi = work.tile([128, T * 16], mybir.dt.bfloat16, tag="hi")
                nc.vector.tensor_copy(hi[:], sc32[:])
                lo = work.tile([128, T * 16], mybir.dt.bfloat16, tag="lo")
                nc.vector.tensor_tensor(lo[:], sc32[:], hi[:],
                                        mybir.AluOpType.subtract)
                parts = (hi, lo)
            else:
                scaled = work.tile([128, T * 16], sel_dt, tag="scaled")
                nc.vector.tensor_tensor(
                    scaled[:].rearrange("p (c f) -> p c f", f=16),
                    xt[:].rearrange("p (c f) -> p c f", f=16),
                    w_bc, mybir.AluOpType.mult)
                parts = (scaled,)

            sel = work.tile([128, T * W], sel_dt, tag="sel")
            nc.vector.tensor_tensor(
                sel[:].rearrange("p (c j) -> p c j", j=W),
                rkey[:, t * T:(t + 1) * T].unsqueeze(2).broadcast_to([128, T, W]),
                iota[:].unsqueeze(1).broadcast_to([128, T, W]),
                mybir.AluOpType.is_equal)

            pt = psum.tile([128, 16], mybir.dt.float32)
            for c in range(T):
                b = c // cb
                ph = c % cb
                for pi, part in enumerate(parts):
                    nc.tensor.matmul(
                        out=pt[32 * b:32 * b + W, :],
                        lhsT=sel[:, W * c:W * (c + 1)],
                        rhs=part[:, 16 * c:16 * (c + 1)],
                        start=(ph == 0 and pi == 0),
                        stop=(ph == cb - 1 and pi == len(parts) - 1))
            nc.scalar.copy(out_sbuf[:96, 16 * t:16 * (t + 1)], pt[:96, :])

        nc.sync.dma_start(
            out_dram.rearrange("(g p) f -> p g f", p=96),
            out_sbuf[:96].rearrange("p (g f) -> p g f", f=16))

    nc.compile()
    return nc


def _prepare_inputs(senders, lengths, receivers, cb: int):
    """Counting-sort + bucket-pad + tile-transpose. Returns in_maps (8 dicts)."""
    cap = cb * 128
    T = BPG * cb
    nchunks = GROUPS * T

    recv = np.ascontiguousarray(np.asarray(receivers).astype(np.int64))
    order = np.argsort(recv, kind="stable").astype(np.int64)
    rs = recv[order]                         # sorted receivers
    rk_sorted = (rs % W).astype(np.float32)  # node-in-bucket key
    gbucket_counts = np.bincount((rs // W).astype(np.int64),
                                 minlength=N_RBUCKETS_TOT)
    starts = np.concatenate([[0], np.cumsum(gbucket_counts)[:-1]])

    senders_ext = np.concatenate(
        [np.asarray(senders, dtype=np.float32), np.zeros((1, F), np.float32)])
    len_ext = np.concatenate(
        [np.asarray(lengths, dtype=np.float32).reshape(-1),
         np.full(1, 6.0, np.float32)])
    rk_ext = np.concatenate([rk_sorted, np.zeros(1, np.float32)])

    iota32 = np.tile(np.arange(W, dtype=np.float32), (128, 1))
    arange_cap = np.arange(cap, dtype=np.int64)

    in_maps = []
    for k in range(N_CORES):
        bidx = RBUCKETS * k + np.arange(BUCKETS_CORE)
        slot_real = (np.arange(BUCKETS_CORE) < RBUCKETS) & (bidx < N_RBUCKETS_TOT)
        cnt = np.where(slot_real, gbucket_counts[np.minimum(bidx, N_RBUCKETS_TOT - 1)], 0)
        st = np.where(slot_real, starts[np.minimum(bidx, N_RBUCKETS_TOT - 1)], 0)
        if cnt.max() > cap:
            raise ValueError(f"bucket overflow: {cnt.max()} > {cap}")
        src = st[:, None] + arange_cap[None, :]          # [393, cap] sorted pos
        valid = arange_cap[None, :] < cnt[:, None]
        srcc = np.minimum(src, E - 1)
        edge_ids = np.where(valid, order[srcc], E)       # E -> pad row
        sort_ids = np.where(valid, srcc, E)              # for sorted-keyed arrays

        x_pad = senders_ext[edge_ids.reshape(-1)]        # [393*cap, 16]
        l_pad = len_ext[edge_ids.reshape(-1)]            # pad row E -> 6.0
        r_pad = rk_ext[sort_ids.reshape(-1)]             # pad row E -> 0.0

        x_t = x_pad.reshape(GROUPS, T, 128, 16).transpose(0, 2, 1, 3).reshape(
            GROUPS, 128, T * 16)
        len_t = np.ascontiguousarray(l_pad.reshape(nchunks, 128).T)
        rkey_t = np.ascontiguousarray(r_pad.reshape(nchunks, 128).T)
        in_maps.append({
            "x_t": np.ascontiguousarray(x_t, dtype=np.float32),
            "len_t": len_t.astype(np.float32),
            "rkey_t": rkey_t.astype(np.float32),
            "iota32": iota32,
        })
    return in_maps


def _get_program(cb: int, mode: str, reps: int = 1):
    key = (cb, mode, reps)
    if key not in _CACHE:
        _CACHE[key] = _build_program(cb, mode, reps)
    return _CACHE[key]


def _pick_cb(receivers):
    """Smallest chunks-per-bucket that fits the densest 32-node bucket."""
    counts = np.bincount(np.asarray(receivers).astype(np.int64) // W,
                         minlength=N_RBUCKETS_TOT)
    return max(6, int(-(-counts.max() // 128)))


def _run(inputs, cb=None, mode="comp", trace=False, **run_kwargs):
    from concourse.bass_utils import run_bass_kernel_spmd

    if cb is None:
        cb = _pick_cb(inputs["receivers"])
    in_maps = _prepare_inputs(inputs["senders"], inputs["lengths"],
                              inputs["receivers"], cb)
    nc = _get_program(cb, mode)
    try:
        res = run_bass_kernel_spmd(nc, in_maps, core_ids=list(range(N_CORES)),
                                   trace=trace, **run_kwargs)
    except Exception:
        # transient NRT device wedges have been observed; one retry
        res = run_bass_kernel_spmd(nc, in_maps, core_ids=list(range(N_CORES)),
                                   trace=trace, **run_kwargs)
    out_full = np.zeros((E, F), np.float32)
    for k in range(N_CORES):
        nk = min(NODES_PER_CORE, N_NODES - NODES_PER_CORE * k)
        if nk <= 0:
            continue
        out_full[NODES_PER_CORE * k:NODES_PER_CORE * k + nk] = \
            res.results[k]["out"][:nk]
    return out_full, res


def kernel(senders, lengths, vectors, receivers):
    out, _ = _run({"senders": senders, "lengths": lengths,
                   "receivers": receivers})
    return out
